# revision 61
# baseline (speedup 1.0000x reference)
"""Trainium2 Bass kernel for nn_AttentionModule_16398185136487.

Math (the reference reduces to this — its trailing softmax is over a size-1
axis, i.e. ones):
  out = concat([x34, a_x4, x43, b_x3], axis=1)            # (8, 512, 32, 32)
  block(qs, ks, v) = gate(qs, ks) * (w128@wv @ x_v + w128@bv) + b128
  gate(qs, ks)[b, hw] = softmax_hw( (1/8) sum_{kb} max_{khw}
                                    (Q_qs[b,hw] . K_ks[kb,khw]) / 16 )

Sharding: core j owns batch image j (its 1024 query pixels for both the x4
and x3 streams) — the per-image softmax is then fully core-local; no
collectives.  The K tensors (all 16 key images) are computed replicated on
every core from the full x4/x3.

Engine plan per core (~293 us, rel err ~3.1e-3):
  - Q/K convs and the 1024 score matmuls in bf16 (fp32 PSUM accumulate,
    contiguous start/stop pairs over C=256 — interleaved accumulation
    groups give wrong results on this hardware).  V path in bf16 with the
    two convs fused host-side (w128@wv).
  - K-conv inputs ride the sync HWDGE queue as one 1 MB DMA per ci half
    per image pair, prefetched one group ahead; Q-conv inputs ride the
    otherwise-idle SWDGE queues.  The first K pair is emitted before the
    Q conv so the PE starts as soon as 1 MB (not 2.5 MB) has landed.
  - ALL PSUM tiles are one bank ([128,512], ps pool bufs=6): halving
    the rotation quantum vs [128,1024]x3 removed ~9 us of PE stalls --
    the PE no longer waits for whole two-bank tiles to clear consumers.
  - exact per-image max over keys, split across the PSUM-capable engines:
    ScalarE stages the first key half from PSUM to SBUF and a custom DVE
    op (TTMAX_REDUCE — the ISA-level TENSOR_TENSOR_REDUCE crashes this
    hardware) consumes the staged half and the other PSUM half in one
    512-wide pass using both DVE read ports.
  - per-image softmax without max-subtraction (logits are O(1)), gates
    broadcast to 128 partitions via K=1 PE matmuls, and a second custom
    DVE op (GMUL_BIAS) applies out = gate_row * V * (1/S) + b128 in one
    pass.  Gate pair (aa, ba) runs overlapped with the x3-key score
    groups; only (ab, bb) is a tail.

Avenues measured and rejected (see session notes): fp8 DoubleRow scores
(the 2x rate only engages for 64-partition outputs, which doubles every
downstream reduce cost; the 128-out shape runs at 1x), a shifted
log-sum-exp max replacement on ScalarE (the fp32 exp range and the ACT
table's ln() breaking above ~e^44 cap the usable beta, leaving 2-4% gate
error), and distributing the replicated K conv via AllGather (the
collective takes >100 us and stalls the PE mid-schedule).  An fp8
DoubleRow K conv (64-out fast shape + partition-shift DMAs for the
misaligned cout chunks) also regressed: the PE->convert->shift-DMA chain
stalls the PE at every group boundary and re-triggers HAM throttling.
PE warm-up dummies (to pre-release the HAM clock gate during the input
DMA wait) were also a wash: ident-based dummies sit behind gpsimd queue
work and start too late; memset-based dummies measured +0.7 us.  Finer
startup tweaks (chunked first-group loads, kconv(0)+kconv(1) ahead of
the Q conv, tail out-DMA split across queues) measured +2 us — the
~16 us head is dominated by the fixed NEFF preamble, not DMA order.
"""

import numpy as np
import ml_dtypes

B = 8
C = 256
HW = 1024          # 32*32
BHW = B * HW       # 8192
NCORES = 8



_CACHE = {}


def _ref_ttmax(in0, in1, c0, c1, c2):
    b = np.maximum(in0.astype(np.float32), in1.astype(np.float32))
    return b, np.maximum(c0, b.reshape(b.shape[0], -1).max(axis=-1, keepdims=True))


def _ref_gmul_bias(in0, in1, c0, c1, c2):
    return (in0.astype(np.float32) * in1 * c1 + c0).astype(np.float32)


def _get_custom_ops():
    """Register two custom DVE microcode ops (the ISA-level
    TENSOR_TENSOR_REDUCE crashes this hardware, so we ship our own):

      TTMAX_REDUCE: out = max(in0, in1); accum_out = max(s0, max_k out)
      GMUL_BIAS:    out = in0 * in1 * s1 + s0     (s0, s1 per-partition APs)
    """
    if "ops" in _CACHE:
        return _CACHE["ops"]
    import concourse.dve_ops as dve_ops
    from concourse.dve_ops import DveOp
    from concourse.dve_spec import Spec, Src0, Src1, C0, C1, maxx, lower
    from concourse.dve_uop import DveOpSpec

    def register(name, spec):
        for op in dve_ops.OPS:
            if op.name == name:
                return op
        shas = {}
        for ver in ("v3", "v4"):
            shas[ver] = DveOpSpec(name=name, opcode=1,
                                  uops=lower(spec, ver=ver),
                                  rd1_en=True).sha(ver)
        op = DveOp(name, spec, subdim=False, uops_sha=shas)
        dve_ops.OPS.append(op)
        dve_ops.CUSTOM_DVE_SPECS[op.name] = op.spec
        dve_ops._SUB_OPCODE_FOR_NAME[op.name] = (
            dve_ops._CUSTOM_DVE_ROW_BASE + len(dve_ops.OPS) - 1)
        assert max(dve_ops._SUB_OPCODE_FOR_NAME.values()) < 0x20
        return op

    ttmax = register("TTMAX_REDUCE",
                     Spec(body=maxx(Src0, Src1), accum=maxx, accum_init=C0,
                          reference=_ref_ttmax))
    gmul = register("GMUL_BIAS",
                    Spec(body=Src0 * Src1 * C1 + C0,
                         reference=_ref_gmul_bias))
    _CACHE["ops"] = (ttmax, gmul)
    return _CACHE["ops"]


def _build_nc():
    from contextlib import ExitStack

    import concourse.bass as bass
    import concourse.mybir as mybir
    import concourse.tile as tile
    from concourse import bacc
    from concourse.masks import make_identity

    f32 = mybir.dt.float32
    bf16 = mybir.dt.bfloat16
    fp8 = mybir.dt.float8e4
    AX = mybir.AxisListType.X
    Exp = mybir.ActivationFunctionType.Exp
    Ln = mybir.ActivationFunctionType.Ln
    Ident = mybir.ActivationFunctionType.Identity
    DR = mybir.MatmulPerfMode.DoubleRow
    MUL = mybir.AluOpType.mult
    ADD = mybir.AluOpType.add

    ttmax, gmul = _get_custom_ops()
    nc = bacc.Bacc("TRN2", target_bir_lowering=False, debug=False,
                   enable_asserts=False, num_devices=NCORES)

    # DRAM I/O (per core)
    x4b_ap = nc.dram_tensor("x4b", (C, BHW), bf16, kind="ExternalInput").ap()
    x3b_ap = nc.dram_tensor("x3b", (C, BHW), bf16, kind="ExternalInput").ap()
    xq_ap = nc.dram_tensor("xq", (C, 2 * HW), bf16, kind="ExternalInput").ap()
    xv_ap = nc.dram_tensor("xv", (C, 2 * HW), bf16, kind="ExternalInput").ap()
    wqT_ap = nc.dram_tensor("wqT", (C, C), bf16, kind="ExternalInput").ap()
    wkT_ap = nc.dram_tensor("wkT", (C, C), bf16, kind="ExternalInput").ap()
    wvT_ap = nc.dram_tensor("wvT", (C, 128), bf16, kind="ExternalInput").ap()
    bq_ap = nc.dram_tensor("bq", (C, 1), f32, kind="ExternalInput").ap()
    bk_ap = nc.dram_tensor("bk", (C, 1), f32, kind="ExternalInput").ap()
    bvb_ap = nc.dram_tensor("bvb", (128, 1), f32, kind="ExternalInput").ap()
    b128_ap = nc.dram_tensor("b128", (128, 1), f32, kind="ExternalInput").ap()
    out_ap = nc.dram_tensor("out", (512, HW), f32, kind="ExternalOutput").ap()

    DEBUG = _CACHE.get("debug", False)
    if DEBUG:
        mdbg_ap = nc.dram_tensor("m_dbg", (128, 256), f32,
                                 kind="ExternalOutput").ap()
        qdbg_ap = nc.dram_tensor("q_dbg", (128, 4 * HW), bf16,
                                 kind="ExternalOutput").ap()
        kdbg_ap = nc.dram_tensor("k_dbg", (128, 2 * BHW), bf16,
                                 kind="ExternalOutput").ap()
        kbdbg_ap = nc.dram_tensor("kb_dbg", (128, 2 * BHW), bf16,
                                  kind="ExternalOutput").ap()

    SCALE_EFF = (1.0 / 16.0) / 8.0        # /sqrt(C), /8 mean

    with tile.TileContext(nc) as tc:
        with ExitStack() as ctx:
            const = ctx.enter_context(tc.tile_pool(name="const", bufs=1))
            xs = ctx.enter_context(tc.tile_pool(name="xs", bufs=8))
            ps_pool = ctx.enter_context(
                tc.tile_pool(name="ps", bufs=6, space="PSUM"))
            gps_pool = ctx.enter_context(
                tc.tile_pool(name="gps", bufs=2, space="PSUM"))
            scr = ctx.enter_context(tc.tile_pool(name="scr", bufs=3))
            gp = ctx.enter_context(tc.tile_pool(name="gp", bufs=2))
            fin = ctx.enter_context(tc.tile_pool(name="fin", bufs=2))

            # ---- weights / constants (queue-critical first) ----
            wq_sb, bq_sb, xq_sb = [], [], []
            for ci in range(2):
                w = const.tile([128, C], bf16, tag=f"wq{ci}", name=f"wq{ci}")
                nc.gpsimd.dma_start(w[:], wqT_ap[ci * 128:(ci + 1) * 128, :])
                wq_sb.append(w)
                t = const.tile([128, 2 * HW], bf16, tag=f"xq{ci}",
                               name=f"xq{ci}")
                xq_sb.append(t)
                b = const.tile([128, 1], f32, tag=f"bq{ci}", name=f"bq{ci}")
                nc.gpsimd.dma_start(b[:], bq_ap[ci * 128:(ci + 1) * 128, :])
                bq_sb.append(b)
            wk_sb, bk_sb = [], []
            for ci in range(2):
                w = const.tile([128, C], bf16, tag=f"wk{ci}", name=f"wk{ci}")
                nc.scalar.dma_start(w[:], wkT_ap[ci * 128:(ci + 1) * 128, :])
                wk_sb.append(w)
                b = const.tile([128, 1], f32, tag=f"bk{ci}", name=f"bk{ci}")
                nc.gpsimd.dma_start(b[:], bk_ap[ci * 128:(ci + 1) * 128, :])
                bk_sb.append(b)
            bvb_sb = const.tile([128, 1], f32, tag="bvb", name="bvb")
            nc.gpsimd.dma_start(bvb_sb[:], bvb_ap[:, :])
            b128_sb = const.tile([128, 1], f32, tag="b128", name="b128")
            nc.gpsimd.dma_start(b128_sb[:], b128_ap[:, :])

            ones_row = const.tile([1, 128], f32, tag="ones_row", name="ones_row")
            nc.vector.memset(ones_row[:], 1.0)
            # bf16 twin for the gate-broadcast matmuls: fp32 matmul is 4
            # cyc/row, so each [128,512] broadcast costs 853 ns vs 213 bf16
            ones_row_b = const.tile([1, 128], bf16, tag="ones_rb",
                                    name="ones_rb")
            nc.vector.memset(ones_row_b[:], 1.0)
            ones_col = const.tile([128, 1], f32, tag="ones_col", name="ones_col")
            nc.vector.memset(ones_col[:], 1.0)
            ident = const.tile([128, 128], f32, tag="ident", name="ident")
            make_identity(nc, ident[:])


            # bf16 feature residents (2D tiles viewed as [128, co, pix])
            q8 = const.tile([128, 2 * 2 * HW], bf16, tag="q8", name="q8")
            ka8 = const.tile([128, 2 * BHW], bf16, tag="ka8", name="ka8")
            kb8 = const.tile([128, 2 * BHW], bf16, tag="kb8", name="kb8")
            q3 = q8.rearrange("p (s n) -> p s n", s=2)
            ka3 = ka8.rearrange("p (s n) -> p s n", s=2)
            kb3 = kb8.rearrange("p (s n) -> p s n", s=2)
            va_sb = const.tile([128, HW], f32, tag="va", name="va")
            vb_sb = const.tile([128, HW], f32, tag="vb", name="vb")

            # per-(q-tile, image) exact maxes: m_all col = qs*16 + img
            m_all = const.tile([128, 256], f32, tag="m_all", name="m_all")
            m3 = m_all.rearrange("p (q i) -> p q i", q=16)
            # M pair tiles: cols 0:8 = first gate (qs 0..7), 8:16 = second
            Mka = const.tile([128, 16], f32, tag="Mka", name="Mka")  # (aa, ba)
            Mkb = const.tile([128, 16], f32, tag="Mkb", name="Mkb")  # (ab, bb)

            # ---- Q conv (bf16) ----
            def emit_qconv():
                for n2 in range(2):
                    for co in range(2):
                        for half in range(2):
                            qps = ps_pool.tile([128, 512], f32, tag="ps",
                                               name="qps")
                            n = n2 * 2 + half
                            for ci in range(2):
                                nc.tensor.matmul(
                                    qps[:, :],
                                    wq_sb[ci][:, co * 128:(co + 1) * 128],
                                    xq_sb[ci][:, n * 512:(n + 1) * 512],
                                    start=(ci == 0), stop=(ci == 1))
                            base = co * 2048 + n2 * 1024 + half * 512
                            nc.scalar.activation(
                                q8[:, base:base + 512],
                                qps[:, :], Ident, bias=bq_sb[co][:])

            # ---- K conv for one key-image pair (2048 cols of one stream)
            # inputs batched as one 1 MB DMA per ci half on the (otherwise
            # idle) sync queue -- fewer, larger transfers and no DMA-issue
            # time on the busy scalar engine.
            def prefetch_xt(grp):
                src_ap = x4b_ap if grp < 4 else x3b_ap
                n2 = (grp * 2) % 8
                xt = []
                for ci in range(2):
                    t = xs.tile([128, 2048], bf16, tag="xt", name="xt",
                                bufs=4)
                    # first pair rides two queues so the PE starts sooner
                    eng = nc.sync if (grp == 0 and ci == 0) else (
                        nc.scalar if grp == 0 else nc.sync)
                    eng.dma_start(
                        t[:], src_ap[ci * 128:(ci + 1) * 128,
                                     n2 * 1024:(n2 + 2) * 1024])
                    xt.append(t)
                return xt

            def emit_kconv_pair(grp, xt):
                k8 = ka8 if grp < 4 else kb8
                n2 = (grp * 2) % 8
                for par in range(2):
                    for co in range(2):
                        for half in range(2):
                            kps = ps_pool.tile([128, 512], f32, tag="ps",
                                               name="kps")
                            for ci in range(2):
                                nc.tensor.matmul(
                                    kps[:, :],
                                    wk_sb[ci][:, co * 128:(co + 1) * 128],
                                    xt[ci][:, par * 1024 + half * 512:
                                           par * 1024 + (half + 1) * 512],
                                    start=(ci == 0), stop=(ci == 1))
                            base = (co * BHW + (n2 + par) * 1024
                                    + half * 512)
                            nc.scalar.activation(k8[:, base:base + 512],
                                                 kps[:, :], Ident,
                                                 bias=bk_sb[co][:])

            # ---- V conv (fp32, fused weights) ----
            def emit_vconv():
                wv_sb, xv_sb = [], []
                for ci in range(2):
                    w = const.tile([128, 128], bf16, tag=f"wv{ci}", name=f"wv{ci}")
                    nc.gpsimd.dma_start(w[:], wvT_ap[ci * 128:(ci + 1) * 128, :])
                    wv_sb.append(w)
                    t = const.tile([128, 2 * HW], bf16, tag=f"xv{ci}",
                                   name=f"xv{ci}")
                    nc.gpsimd.dma_start(t[:], xv_ap[ci * 128:(ci + 1) * 128, :])
                    xv_sb.append(t)
                for st, v_sb in ((0, va_sb), (1, vb_sb)):
                    for half in range(2):
                        vps = ps_pool.tile([128, 512], f32, tag="ps",
                                           name="vps")
                        for ci in range(2):
                            nc.tensor.matmul(
                                vps[:, :],
                                wv_sb[ci][:, :],
                                xv_sb[ci][:, st * HW + half * 512:
                                           st * HW + (half + 1) * 512],
                                start=(ci == 0), stop=(ci == 1))
                        nc.scalar.activation(
                            v_sb[:, half * 512:(half + 1) * 512],
                            vps[:, :], Ident, bias=bvb_sb[:])

            # ---- scores for one (q-tile, key-image-pair) ----
            # bf16 matmuls, fp32 PSUM accumulate over C=256 (contiguous
            # start/stop pairs; interleaved accumulation groups give wrong
            # results on this hardware).
            # Exact per-image max, engine-balanced two ways:
            #   most images: ScalarE stages keys 0:512 from PSUM to SBUF,
            #     then the TTMAX_REDUCE DVE op consumes the staged half and
            #     the PSUM half in one 512-wide pass (both DVE read ports).
            #   a few images: VectorE native reduce_max over the full 1024
            #     PSUM columns, no scalar stage.
            def emit_scores(qs, grp):
                k3 = ka3 if grp < 4 else kb3
                n2 = (grp * 2) % 8
                qcol = qs * 128

                def score_mms(tdst, kimg_col, kh):
                    for ci in range(2):
                        nc.tensor.matmul(
                            tdst[:, :],
                            q3[:, ci, qcol:qcol + 128],
                            k3[:, ci, kimg_col + kh * 512:
                               kimg_col + kh * 512 + 512],
                            start=(ci == 0), stop=(ci == 1))

                for par in range(2):
                    img = grp * 2 + par
                    kimg_col = (n2 + par) * 1024
                    tA = ps_pool.tile([128, 512], f32, tag="ps", name="tA")
                    score_mms(tA, kimg_col, 0)
                    tB = ps_pool.tile([128, 512], f32, tag="ps", name="tB")
                    score_mms(tB, kimg_col, 1)
                    mcol = m_all[:, qs * 16 + img:qs * 16 + img + 1]
                    cp = scr.tile([128, 512], f32, tag="cp", name="cp",
                                  bufs=8)
                    nc.scalar.copy(cp[:], tA[:, :])
                    sc = scr.tile([128, 512], f32, tag="sc", name="sc",
                                  bufs=8)
                    nc.vector._custom_dve(
                        ttmax, out=sc[:], in0=tB[:, :], in1=cp[:],
                        s0=-3.0e38, accum_out=mcol)

            # ---- batched softmax + apply for a pair of gates ----
            def emit_gate_single(M8, v_sb, blk, tagp):
                # single-gate variant of emit_gate_pair: M8 is [128, 8]
                # (this gate's 8 q-tiles); emitted as soon as its half of
                # the final score group is consumed, hiding the serial
                # softmax chain under the other half's scores.
                E1 = gp.tile([128, 8], f32, tag=f"E{tagp}", name="E1")
                nc.scalar.activation(E1[:], M8, Exp, bias=0.0,
                                     scale=SCALE_EFF)
                sr = gp.tile([128, 1], f32, tag=f"sr{tagp}", name="sr")
                nc.vector.reduce_sum(sr[:], E1[:], axis=AX)
                sum_ps = gps_pool.tile([128, 512], f32, tag="gps",
                                       name="sum_ps")
                nc.tensor.matmul(sum_ps[0:1, 0:1], sr[:], ones_col[:],
                                 start=True, stop=True)
                rec = gp.tile([1, 1], f32, tag=f"rec{tagp}", name="rec")
                nc.vector.reciprocal(rec[:], sum_ps[0:1, 0:1])
                bc = gps_pool.tile([128, 512], f32, tag="gps", name="bc")
                nc.tensor.matmul(bc[:, 0:1], ones_row[:], rec[:],
                                 start=True, stop=True)
                rsb = gp.tile([128, 1], f32, tag=f"rsb{tagp}", name="rsb")
                nc.scalar.copy(rsb[:], bc[:, 0:1])
                tpe = gps_pool.tile([128, 512], f32, tag="gps", name="tpe")
                nc.tensor.transpose(tpe[0:8, 0:128], E1[:], ident[:])
                et = gp.tile([8, 128], bf16, tag=f"et{tagp}", name="et")
                nc.scalar.copy(et[:], tpe[0:8, 0:128])
                grow = gp.tile([1, 1024], bf16, tag=f"grow{tagp}",
                               name="grow")
                nc.sync.dma_start(grow.rearrange("a (t p) -> a t p", t=8),
                                  et[:])
                out_t = fin.tile([128, HW], f32, tag="out_t", name="out_t")
                for half in range(2):
                    gb = gps_pool.tile([128, 512], f32, tag="gps", name="gb")
                    nc.tensor.matmul(
                        gb[:, :], ones_row_b[:],
                        grow[0:1, half * 512:(half + 1) * 512],
                        start=True, stop=True)
                    nc.vector._custom_dve(
                        gmul, out=out_t[:, half * 512:(half + 1) * 512],
                        in0=gb[:, :],
                        in1=v_sb[:, half * 512:(half + 1) * 512],
                        s0=b128_sb[:], s1=rsb[:])
                eng = nc.sync if blk % 2 == 0 else nc.scalar
                eng.dma_start(out_ap[blk * 128:(blk + 1) * 128, :], out_t[:])

            def emit_gate_pair_a(Mpair, tagp):
                E2 = gp.tile([128, 16], f32, tag=f"E2{tagp}", name="E2")
                nc.scalar.activation(E2[:], Mpair[:], Exp, bias=0.0,
                                     scale=SCALE_EFF)
                sr = gp.tile([128, 2], f32, tag=f"sr{tagp}", name="sr")
                nc.vector.reduce_sum(
                    sr[:], E2.rearrange("p (g k) -> p g k", g=2), axis=AX)
                return E2, sr

            def emit_gate_pair(ab, specs, tagp):
                E2, sr = ab
                sum_ps = gps_pool.tile([128, 512], f32, tag="gps", name="sum_ps")
                nc.tensor.matmul(sum_ps[0:2, 0:1], sr[:], ones_col[:],
                                 start=True, stop=True)
                rec2 = gp.tile([2, 1], f32, tag=f"rec{tagp}", name="rec2")
                nc.vector.reciprocal(rec2[:], sum_ps[0:2, 0:1])
                tp = gps_pool.tile([128, 512], f32, tag="gps", name="tp")
                nc.tensor.transpose(tp[0:1, 0:2], rec2[:], ident[0:2, 0:2])
                recT = gp.tile([1, 2], f32, tag=f"recT{tagp}", name="recT")
                nc.scalar.copy(recT[:], tp[0:1, 0:2])
                bc = gps_pool.tile([128, 512], f32, tag="gps", name="bc")
                nc.tensor.matmul(bc[:, 0:2], ones_row[:], recT[:],
                                 start=True, stop=True)
                rsb2 = gp.tile([128, 2], f32, tag=f"rsb{tagp}", name="rsb2")
                nc.scalar.copy(rsb2[:], bc[:, 0:2])
                # transpose E (128,16) -> (16,128), flatten to a (1,2048) row
                tpe = gps_pool.tile([128, 512], f32, tag="gps", name="tpe")
                nc.tensor.transpose(tpe[0:16, 0:128], E2[:], ident[:])
                et = gp.tile([16, 128], bf16, tag=f"et{tagp}", name="et")
                nc.scalar.copy(et[:], tpe[0:16, 0:128])
                grow = gp.tile([1, 2048], bf16, tag=f"grow{tagp}",
                               name="grow")
                nc.sync.dma_start(grow.rearrange("a (t p) -> a t p", t=16),
                                  et[:])
                for gidx, (v_sb, blk) in enumerate(specs):
                    out_t = fin.tile([128, HW], f32, tag="out_t", name="out_t")
                    for half in range(2):
                        gb = gps_pool.tile([128, 512], f32, tag="gps",
                                           name="gb")
                        nc.tensor.matmul(
                            gb[:, :], ones_row_b[:],
                            grow[0:1, gidx * 1024 + half * 512:
                                 gidx * 1024 + (half + 1) * 512],
                            start=True, stop=True)
                        nc.vector._custom_dve(
                            gmul, out=out_t[:, half * 512:(half + 1) * 512],
                            in0=gb[:, :],
                            in1=v_sb[:, half * 512:(half + 1) * 512],
                            s0=b128_sb[:], s1=rsb2[:, gidx:gidx + 1])
                    eng = nc.sync if gidx == 0 else nc.scalar
                    eng.dma_start(out_ap[blk * 128:(blk + 1) * 128, :],
                                  out_t[:])

            # ---- main schedule ----
            # kconv for the first image pair is emitted BEFORE the Q conv:
            # its inputs are a single 512 KB stream, so the PE starts ~15 us
            # earlier than if it had to wait for the 2.5 MB Q-conv inputs.
            xt_next = prefetch_xt(0)
            # Q-conv inputs follow the first K pair on the two HWDGE queues:
            # the PE's first kconv matmuls start after only ~0.5 MB, and xq
            # lands just in time for the Q conv right behind them.
            for nq in range(2):
                for ci in range(2):
                    eng = nc.sync if ci == 0 else nc.scalar
                    eng.dma_start(
                        xq_sb[ci][:, nq * HW:(nq + 1) * HW],
                        xq_ap[ci * 128:(ci + 1) * 128, nq * HW:(nq + 1) * HW])
            for grp in range(4):               # x4-stream key images 0..7
                xt_cur, xt_next = xt_next, None
                emit_kconv_pair(grp, xt_cur)
                xt_next = prefetch_xt(grp + 1)
                if grp == 0:
                    emit_qconv()
                if grp == 2:
                    emit_vconv()
                for qs in range(16):
                    emit_scores(qs, grp)

            def emit_msum(Mdst, lo):
                # all 16 q-tiles at once: sum the 8 per-image maxes
                nc.vector.reduce_sum(
                    Mdst[:], m3[:, :, 2 * lo:2 * lo + 8], axis=AX)

            emit_msum(Mka, 0)
            gate1_ab = emit_gate_pair_a(Mka, "1")
            for grp in range(4, 8):            # x3-stream key images 8..15
                xt_cur, xt_next = xt_next, None
                emit_kconv_pair(grp, xt_cur)
                if grp < 7:
                    xt_next = prefetch_xt(grp + 1)
                for qs in range(16):
                    emit_scores(qs, grp)
                    if grp == 7 and qs == 7:
                        # gate ab (qs 0..7) is fully consumed: emit its
                        # softmax chain now so it overlaps qs 8..15
                        nc.vector.reduce_sum(Mkb[:, 0:8],
                                             m3[:, 0:8, 8:16], axis=AX)
                        emit_gate_single(Mkb[:, 0:8], vb_sb, 2, "2a")
                if grp == 5:
                    # (aa -> block 1, ba -> block 0).  Emitted mid-stream so
                    # its serial softmax chain gets LOW priority and overlaps
                    # the remaining score groups.
                    emit_gate_pair(gate1_ab, [(va_sb, 1), (va_sb, 0)],
                                   "1")
            # gate bb (qs 8..15) is the only tail
            nc.vector.reduce_sum(Mkb[:, 8:16], m3[:, 8:16, 8:16], axis=AX)
            emit_gate_single(Mkb[:, 8:16], vb_sb, 3, "2b")
            if DEBUG:
                nc.gpsimd.dma_start(mdbg_ap[:, :], m_all[:])
                nc.gpsimd.dma_start(qdbg_ap[:, :], q8[:])
                nc.gpsimd.dma_start(kdbg_ap[:, :], ka8[:])
                nc.gpsimd.dma_start(kbdbg_ap[:, :], kb8[:])

    nc.compile()
    return nc


def get_nc():
    if "nc" not in _CACHE:
        _CACHE["nc"] = _build_nc()
    return _CACHE["nc"]


def prepare_in_maps(x4, x3, wq, bq, wk, bk, wv, bv, w128, b128):
    bf16 = ml_dtypes.bfloat16
    x4 = np.asarray(x4, np.float32)
    x3 = np.asarray(x3, np.float32)
    X4 = np.ascontiguousarray(x4.transpose(1, 0, 2, 3).reshape(C, BHW))
    X3 = np.ascontiguousarray(x3.transpose(1, 0, 2, 3).reshape(C, BHW))
    X4b = X4.astype(bf16)
    X3b = X3.astype(bf16)
    wq = np.asarray(wq, np.float32)
    wk = np.asarray(wk, np.float32)
    wv = np.asarray(wv, np.float32)
    w128 = np.asarray(w128, np.float32)
    wqT = np.ascontiguousarray(wq.T).astype(bf16)
    wkT = np.ascontiguousarray(wk.T).astype(bf16)
    wvT = np.ascontiguousarray((w128 @ wv).T).astype(bf16)   # (256, 128)
    bq2 = np.asarray(bq, np.float32).reshape(C, 1)
    bk2 = np.asarray(bk, np.float32).reshape(C, 1)
    bvb = (w128 @ np.asarray(bv, np.float32)).reshape(128, 1).astype(np.float32)
    b128r = np.asarray(b128, np.float32).reshape(128, 1)

    in_maps = []
    for j in range(NCORES):
        sl = slice(j * HW, (j + 1) * HW)
        xq = np.concatenate([X4b[:, sl], X3b[:, sl]], axis=1)
        xv = np.concatenate([X4b[:, sl], X3b[:, sl]], axis=1)
        in_maps.append({
            "x4b": X4b, "x3b": X3b,
            "xq": np.ascontiguousarray(xq),
            "xv": np.ascontiguousarray(xv),
            "wqT": wqT, "wkT": wkT, "wvT": wvT,
            "bq": bq2, "bk": bk2, "bvb": bvb, "b128": b128r,
        })
    return in_maps


def kernel(**inputs):
    from concourse.bass_utils import run_bass_kernel_spmd

    nc = get_nc()
    in_maps = prepare_in_maps(**inputs)
    res = run_bass_kernel_spmd(nc, in_maps, core_ids=list(range(NCORES)))
    out = np.stack([res.results[c]["out"].reshape(512, 32, 32)
                    for c in range(NCORES)])
    return np.ascontiguousarray(out.astype(np.float32))
